# revision 24
# baseline (speedup 1.0000x reference)
"""Trainium2 Bass kernel: feature-attention (dense_transformer).

    score = softmax((q^T @ k) / sqrt(H), axis=-1)   # (B,H,D,D), contraction over S
    out   = score @ v^T                              # (B,H,D,S)

q,k,v: (4,16,4096,128) f32.  B*H = 64 head-pairs sharded 8-per-core across
8 NeuronCores (pure data/head parallelism, no collectives).

The kernel is DMA-bound (fp32 would be 64 MiB/core @ 360 GB/s = 187 us
floor), so all device I/O is fp16: inputs are downcast on the host
(logit error ~ sqrt(S)*2^-11*scale stays ~1e-3, final rel err ~6e-3 vs
the 2e-2 gate), output is written fp16 and upcast on the host.  Traffic
is 32 MiB/core -> ~93 us DMA floor.  fp16 matmuls run at 1 cyc/row.

Per (b,h) pair on-core:
  - q,k,v loaded as [128, 16, 128] halves (seq chunked onto partitions,
    4 KiB/partition contiguous) so score matmuls start after the first
    0.5 MiB lands and DMA dependencies stay fine-grained.
  - score[d,e] = sum_s q[s,d] k[s,e]: 32 accumulating fp16 PE matmuls
    (fp32 PSUM) into one PSUM tile.
  - v^T via PE transpose-mode matmuls (fp16, 1 cyc/row), 4 chunks per
    PSUM bank; PSUM->SBUF copies alternate between ACT and DVE to
    balance engine load.
  - softmax along free axis in fp32: reduce_max (DVE), exp with fused
    row-sum (ACT, fp16 out), reciprocal (DVE); normalization deferred
    to the output eviction.
  - out[d,s] = sum_e exp[e,d] vt[e,s]: 8 fp16 matmuls N=512, scaled by
    1/rowsum on the PSUM->SBUF eviction (DVE tensor_scalar, fp16 out),
    each 512-column block stored immediately (early drain).
"""

import math
import sys
from contextlib import ExitStack

for _p in ("/opt/trn_rl_repo", "/root/.axon_site/_ro/trn_rl_repo"):
    if _p not in sys.path:
        sys.path.insert(0, _p)

import numpy as np

import concourse.bacc as bacc
import concourse.bass as bass
import concourse.tile as tile
from concourse import mybir
from concourse.bass_utils import run_bass_kernel_spmd
from concourse.masks import make_identity

B, H, S, D = 4, 16, 4096, 128
NCORES = 8
PAIRS = (B * H) // NCORES  # 8 (b,h) pairs per core
NJ = S // 512              # 8 output column blocks of 512
SCALE = 1.0 / math.sqrt(H)
F32 = mybir.dt.float32
F16 = mybir.dt.float16


def _build():
    nc = bacc.Bacc(
        "TRN2",
        target_bir_lowering=False,
        debug=False,
        enable_asserts=False,
        num_devices=NCORES,
    )
    q = nc.dram_tensor("q", (PAIRS, S, D), F16, kind="ExternalInput").ap()
    k = nc.dram_tensor("k", (PAIRS, S, D), F16, kind="ExternalInput").ap()
    v = nc.dram_tensor("v", (PAIRS, S, D), F16, kind="ExternalInput").ap()
    out = nc.dram_tensor("out", (PAIRS, D, S), F16, kind="ExternalOutput").ap()

    with tile.TileContext(nc) as tc, ExitStack() as ctx:
        const = ctx.enter_context(tc.tile_pool(name="const", bufs=1))
        qkv = ctx.enter_context(tc.tile_pool(name="qkv", bufs=6))
        big = ctx.enter_context(tc.tile_pool(name="big", bufs=2))
        outp = ctx.enter_context(tc.tile_pool(name="outp", bufs=3))
        small = ctx.enter_context(tc.tile_pool(name="small", bufs=2))
        ps_score = ctx.enter_context(tc.tile_pool(name="ps_score", bufs=2, space="PSUM"))
        ps_vt = ctx.enter_context(tc.tile_pool(name="ps_vt", bufs=2, space="PSUM"))
        ps_pt = ctx.enter_context(tc.tile_pool(name="ps_pt", bufs=1, space="PSUM"))
        ps_out = ctx.enter_context(tc.tile_pool(name="ps_out", bufs=3, space="PSUM"))

        ident = const.tile([128, 128], F16)
        make_identity(nc, ident)

        for p in range(PAIRS):
            # ---- loads: (4096,128) f16 in halves; partition p_ holds rows
            # s = p_*32 + jj, jj = h*16 + j in [0,32).  The score contraction
            # is order-independent so any chunk order works as long as q and
            # k share the mapping; v's transpose un-permutes explicitly.
            # v first: its PE transposes fill the pipe while q,k stream in,
            # which shortens the post-load critical chain of the final pair.
            v_sb = qkv.tile([128, 32, 128], F16, tag="v")
            nc.sync.dma_start(out=v_sb, in_=v[p].rearrange("(s j) d -> s j d", s=128))
            q_sb = qkv.tile([128, 32, 128], F16, tag="q")
            k_sb = qkv.tile([128, 32, 128], F16, tag="k")
            nc.sync.dma_start(out=q_sb, in_=q[p].rearrange("(s j) d -> s j d", s=128))
            nc.sync.dma_start(out=k_sb, in_=k[p].rearrange("(s j) d -> s j d", s=128))

            # ---- vT[e, s] via PE transpose (4 chunks per PSUM bank).
            # Transposing chunk jj yields [e, p_] columns for s = p_*32 + jj;
            # the copy-out un-permutes into true s-order with a stride-32
            # free-dim write: vt_sb layout is [e, p_, jj] so free pos = s.
            vt_sb = big.tile([128, 128, 32], F16, tag="vt")
            for g in range(8):
                vt_ps = ps_vt.tile([128, 512], F16, tag="vt")
                for i in range(4):
                    jj = 4 * g + i
                    nc.tensor.transpose(
                        vt_ps[:, 128 * i : 128 * (i + 1)], v_sb[:, jj, :], ident
                    )
                nc.scalar.copy(
                    out=vt_sb[:, :, 4 * g : 4 * g + 4],
                    in_=vt_ps.rearrange("e (i s) -> e s i", i=4),
                )

            # ---- score[d,e] = sum_s q[s,d] k[s,e] ----
            score_ps = ps_score.tile([128, 128], F32, tag="score")
            for jj in range(32):
                nc.tensor.matmul(
                    score_ps,
                    q_sb[:, jj, :],
                    k_sb[:, jj, :],
                    start=(jj == 0),
                    stop=(jj == 31),
                )

            # ---- softmax over free axis e (normalization deferred) ----
            rowmax = small.tile([128, 1], F32, tag="rowmax")
            nc.vector.reduce_max(rowmax, score_ps, axis=mybir.AxisListType.X)
            negb = small.tile([128, 1], F32, tag="negb")
            nc.vector.tensor_scalar_mul(negb, rowmax, -SCALE)
            pexp = small.tile([128, 128], F16, tag="pexp")
            rowsum = small.tile([128, 1], F32, tag="rowsum")
            nc.scalar.activation(
                pexp,
                score_ps,
                mybir.ActivationFunctionType.Exp,
                bias=negb,
                scale=SCALE,
                accum_out=rowsum,
            )
            rinv = small.tile([128, 1], F32, tag="rinv")
            nc.vector.reciprocal(rinv, rowsum)

            # ---- pT[e,d] = exp(score)[d,e]^T ----
            pt_ps = ps_pt.tile([128, 128], F16, tag="pt")
            nc.tensor.transpose(pt_ps, pexp, ident)
            pt_sb = small.tile([128, 128], F16, tag="pt_sb")
            nc.scalar.copy(out=pt_sb, in_=pt_ps)

            # ---- out[d,s] = (1/rowsum[d]) * sum_e pT[e,d] vt[e,s] ----
            # evictions scale by 1/rowsum; one 8 KiB/partition store on the
            # ACT engine's DGE ring so stores can't head-of-line-block loads.
            # The final pair stores in two halves so its store overlaps its
            # own tail evictions (everyone else keeps max-size descriptors).
            nhalf = 2 if p == PAIRS - 1 else 1
            orr = out[p].rearrange("d (u j s) -> u d j s", u=nhalf, j=NJ // nhalf)
            for u in range(nhalf):
                out_sb = outp.tile(
                    [128, NJ // nhalf, 512], F16, tag="out", name=f"out_sb{u}"
                )
                for jh in range(NJ // nhalf):
                    j = u * (NJ // nhalf) + jh
                    out_ps = ps_out.tile([128, 512], F32, tag="out")
                    nc.tensor.matmul(
                        out_ps,
                        pt_sb,
                        vt_sb[:, 16 * j : 16 * (j + 1), :],
                        start=True,
                        stop=True,
                    )
                    nc.vector.tensor_scalar_mul(out_sb[:, jh, :], out_ps, rinv)
                nc.scalar.dma_start(out=orr[u], in_=out_sb)

    nc.compile()
    return nc


_NC = None


def _get_nc():
    global _NC
    if _NC is None:
        _NC = _build()
    return _NC


def _in_maps(q, k, v):
    qf = np.ascontiguousarray(np.asarray(q).reshape(B * H, S, D).astype(np.float16))
    kf = np.ascontiguousarray(np.asarray(k).reshape(B * H, S, D).astype(np.float16))
    vf = np.ascontiguousarray(np.asarray(v).reshape(B * H, S, D).astype(np.float16))
    return [
        {
            "q": qf[i * PAIRS : (i + 1) * PAIRS],
            "k": kf[i * PAIRS : (i + 1) * PAIRS],
            "v": vf[i * PAIRS : (i + 1) * PAIRS],
        }
        for i in range(NCORES)
    ]


def _run(q, k, v, **kwargs):
    nc = _get_nc()
    res = run_bass_kernel_spmd(nc, _in_maps(q, k, v), core_ids=list(range(NCORES)), **kwargs)
    full = np.concatenate([res.results[i]["out"] for i in range(NCORES)], axis=0)
    return full.astype(np.float32).reshape(B, H, D, S), res


def kernel(q, k, v):
    out, _ = _run(q, k, v)
    return out


# revision 25
# speedup vs baseline: 1.0095x; 1.0095x over previous
"""Trainium2 Bass kernel: feature-attention (dense_transformer).

    score = softmax((q^T @ k) / sqrt(H), axis=-1)   # (B,H,D,D), contraction over S
    out   = score @ v^T                              # (B,H,D,S)

q,k,v: (4,16,4096,128) f32.  B*H = 64 head-pairs sharded 8-per-core across
8 NeuronCores (pure data/head parallelism, no collectives).

The kernel is DMA-bound (fp32 would be 64 MiB/core @ 360 GB/s = 187 us
floor), so all device I/O is fp16: inputs are downcast on the host
(logit error ~ sqrt(S)*2^-11*scale stays ~1e-3, final rel err ~6e-3 vs
the 2e-2 gate), output is written fp16 and upcast on the host.  Traffic
is 32 MiB/core -> ~93 us DMA floor.  fp16 matmuls run at 1 cyc/row.

Per (b,h) pair on-core:
  - q,k,v loaded as [128, 16, 128] halves (seq chunked onto partitions,
    4 KiB/partition contiguous) so score matmuls start after the first
    0.5 MiB lands and DMA dependencies stay fine-grained.
  - score[d,e] = sum_s q[s,d] k[s,e]: 32 accumulating fp16 PE matmuls
    (fp32 PSUM) into one PSUM tile.
  - v^T via PE transpose-mode matmuls (fp16, 1 cyc/row), 4 chunks per
    PSUM bank; PSUM->SBUF copies alternate between ACT and DVE to
    balance engine load.
  - softmax along free axis in fp32: reduce_max (DVE), exp with fused
    row-sum (ACT, fp16 out), reciprocal (DVE); normalization deferred
    to the output eviction.
  - out[d,s] = sum_e exp[e,d] vt[e,s]: 8 fp16 matmuls N=512, scaled by
    1/rowsum on the PSUM->SBUF eviction (DVE tensor_scalar, fp16 out),
    each 512-column block stored immediately (early drain).
"""

import math
import sys
from contextlib import ExitStack

for _p in ("/opt/trn_rl_repo", "/root/.axon_site/_ro/trn_rl_repo"):
    if _p not in sys.path:
        sys.path.insert(0, _p)

import numpy as np

import concourse.bacc as bacc
import concourse.bass as bass
import concourse.tile as tile
from concourse import mybir
from concourse.bass_utils import run_bass_kernel_spmd
from concourse.masks import make_identity

B, H, S, D = 4, 16, 4096, 128
NCORES = 8
PAIRS = (B * H) // NCORES  # 8 (b,h) pairs per core
NJ = S // 512              # 8 output column blocks of 512
SCALE = 1.0 / math.sqrt(H)
F32 = mybir.dt.float32
F16 = mybir.dt.float16


def _build():
    nc = bacc.Bacc(
        "TRN2",
        target_bir_lowering=False,
        debug=False,
        enable_asserts=False,
        num_devices=NCORES,
    )
    q = nc.dram_tensor("q", (PAIRS, S, D), F16, kind="ExternalInput").ap()
    k = nc.dram_tensor("k", (PAIRS, S, D), F16, kind="ExternalInput").ap()
    v = nc.dram_tensor("v", (PAIRS, S, D), F16, kind="ExternalInput").ap()
    out = nc.dram_tensor("out", (PAIRS, D, S), F16, kind="ExternalOutput").ap()

    with tile.TileContext(nc) as tc, ExitStack() as ctx:
        const = ctx.enter_context(tc.tile_pool(name="const", bufs=1))
        qkv = ctx.enter_context(tc.tile_pool(name="qkv", bufs=6))
        big = ctx.enter_context(tc.tile_pool(name="big", bufs=2))
        outp = ctx.enter_context(tc.tile_pool(name="outp", bufs=3))
        small = ctx.enter_context(tc.tile_pool(name="small", bufs=2))
        ps_score = ctx.enter_context(tc.tile_pool(name="ps_score", bufs=2, space="PSUM"))
        ps_vt = ctx.enter_context(tc.tile_pool(name="ps_vt", bufs=2, space="PSUM"))
        ps_pt = ctx.enter_context(tc.tile_pool(name="ps_pt", bufs=1, space="PSUM"))
        ps_out = ctx.enter_context(tc.tile_pool(name="ps_out", bufs=3, space="PSUM"))

        ident = const.tile([128, 128], F16)
        make_identity(nc, ident)

        for p in range(PAIRS):
            # ---- loads: (4096,128) f16 in halves; partition p_ holds rows
            # s = p_*32 + jj, jj = h*16 + j in [0,32).  The score contraction
            # is order-independent so any chunk order works as long as q and
            # k share the mapping; v's transpose un-permutes explicitly.
            q_sb = qkv.tile([128, 32, 128], F16, tag="q")
            k_sb = qkv.tile([128, 32, 128], F16, tag="k")
            v_sb = qkv.tile([128, 32, 128], F16, tag="v")
            nc.sync.dma_start(out=q_sb, in_=q[p].rearrange("(s j) d -> s j d", s=128))
            nc.sync.dma_start(out=k_sb, in_=k[p].rearrange("(s j) d -> s j d", s=128))
            nc.sync.dma_start(out=v_sb, in_=v[p].rearrange("(s j) d -> s j d", s=128))

            # ---- score[d,e] = sum_s q[s,d] k[s,e] ----
            score_ps = ps_score.tile([128, 128], F32, tag="score")
            for jj in range(32):
                nc.tensor.matmul(
                    score_ps,
                    q_sb[:, jj, :],
                    k_sb[:, jj, :],
                    start=(jj == 0),
                    stop=(jj == 31),
                )

            # ---- vT[e, s] via PE transpose (4 chunks per PSUM bank).
            # Transposing chunk jj yields [e, p_] columns for s = p_*32 + jj;
            # the copy-out un-permutes into true s-order with a stride-32
            # free-dim write: vt_sb layout is [e, p_, jj] so free pos = s.
            vt_sb = big.tile([128, 128, 32], F16, tag="vt")
            for g in range(8):
                vt_ps = ps_vt.tile([128, 512], F16, tag="vt")
                for i in range(4):
                    jj = 4 * g + i
                    nc.tensor.transpose(
                        vt_ps[:, 128 * i : 128 * (i + 1)], v_sb[:, jj, :], ident
                    )
                nc.scalar.copy(
                    out=vt_sb[:, :, 4 * g : 4 * g + 4],
                    in_=vt_ps.rearrange("e (i s) -> e s i", i=4),
                )

            # ---- softmax over free axis e (normalization deferred) ----
            rowmax = small.tile([128, 1], F32, tag="rowmax")
            nc.vector.reduce_max(rowmax, score_ps, axis=mybir.AxisListType.X)
            negb = small.tile([128, 1], F32, tag="negb")
            nc.vector.tensor_scalar_mul(negb, rowmax, -SCALE)
            pexp = small.tile([128, 128], F16, tag="pexp")
            rowsum = small.tile([128, 1], F32, tag="rowsum")
            nc.scalar.activation(
                pexp,
                score_ps,
                mybir.ActivationFunctionType.Exp,
                bias=negb,
                scale=SCALE,
                accum_out=rowsum,
            )
            rinv = small.tile([128, 1], F32, tag="rinv")
            nc.vector.reciprocal(rinv, rowsum)

            # ---- pT[e,d] = exp(score)[d,e]^T ----
            pt_ps = ps_pt.tile([128, 128], F16, tag="pt")
            nc.tensor.transpose(pt_ps, pexp, ident)
            pt_sb = small.tile([128, 128], F16, tag="pt_sb")
            nc.scalar.copy(out=pt_sb, in_=pt_ps)

            # ---- out[d,s] = (1/rowsum[d]) * sum_e pT[e,d] vt[e,s] ----
            # evictions scale by 1/rowsum; one 8 KiB/partition store on the
            # ACT engine's DGE ring so stores can't head-of-line-block loads.
            # The final pair stores in two halves so its store overlaps its
            # own tail evictions (everyone else keeps max-size descriptors).
            nhalf = 2 if p == PAIRS - 1 else 1
            orr = out[p].rearrange("d (u j s) -> u d j s", u=nhalf, j=NJ // nhalf)
            for u in range(nhalf):
                out_sb = outp.tile(
                    [128, NJ // nhalf, 512], F16, tag="out", name=f"out_sb{u}"
                )
                for jh in range(NJ // nhalf):
                    j = u * (NJ // nhalf) + jh
                    out_ps = ps_out.tile([128, 512], F32, tag="out")
                    nc.tensor.matmul(
                        out_ps,
                        pt_sb,
                        vt_sb[:, 16 * j : 16 * (j + 1), :],
                        start=True,
                        stop=True,
                    )
                    nc.vector.tensor_scalar_mul(out_sb[:, jh, :], out_ps, rinv)
                nc.scalar.dma_start(out=orr[u], in_=out_sb)

    nc.compile()
    return nc


_NC = None


def _get_nc():
    global _NC
    if _NC is None:
        _NC = _build()
    return _NC


def _in_maps(q, k, v):
    qf = np.ascontiguousarray(np.asarray(q).reshape(B * H, S, D).astype(np.float16))
    kf = np.ascontiguousarray(np.asarray(k).reshape(B * H, S, D).astype(np.float16))
    vf = np.ascontiguousarray(np.asarray(v).reshape(B * H, S, D).astype(np.float16))
    return [
        {
            "q": qf[i * PAIRS : (i + 1) * PAIRS],
            "k": kf[i * PAIRS : (i + 1) * PAIRS],
            "v": vf[i * PAIRS : (i + 1) * PAIRS],
        }
        for i in range(NCORES)
    ]


def _run(q, k, v, **kwargs):
    nc = _get_nc()
    res = run_bass_kernel_spmd(nc, _in_maps(q, k, v), core_ids=list(range(NCORES)), **kwargs)
    full = np.concatenate([res.results[i]["out"] for i in range(NCORES)], axis=0)
    return full.astype(np.float32).reshape(B, H, D, S), res


def kernel(q, k, v):
    out, _ = _run(q, k, v)
    return out


# revision 29
# speedup vs baseline: 1.0368x; 1.0271x over previous
"""Trainium2 Bass kernel: feature-attention (dense_transformer).

    score = softmax((q^T @ k) / sqrt(H), axis=-1)   # (B,H,D,D), contraction over S
    out   = score @ v^T                              # (B,H,D,S)

q,k,v: (4,16,4096,128) f32.  B*H = 64 head-pairs sharded 8-per-core across
8 NeuronCores (pure data/head parallelism, no collectives).

The kernel is DMA-bound (fp32 would be 64 MiB/core @ 360 GB/s = 187 us
floor), so all device I/O is fp16: inputs are downcast on the host
(logit error ~ sqrt(S)*2^-11*scale stays ~1e-3, final rel err ~6e-3 vs
the 2e-2 gate), output is written fp16 and upcast on the host.  Traffic
is 32 MiB/core -> ~93 us DMA floor.  fp16 matmuls run at 1 cyc/row.

Per (b,h) pair on-core:
  - q,k,v loaded as [128, 16, 128] halves (seq chunked onto partitions,
    4 KiB/partition contiguous) so score matmuls start after the first
    0.5 MiB lands and DMA dependencies stay fine-grained.
  - score[d,e] = sum_s q[s,d] k[s,e]: 32 accumulating fp16 PE matmuls
    (fp32 PSUM) into one PSUM tile.
  - v^T via PE transpose-mode matmuls (fp16, 1 cyc/row), 4 chunks per
    PSUM bank; PSUM->SBUF copies alternate between ACT and DVE to
    balance engine load.
  - softmax along free axis in fp32: reduce_max (DVE), exp with fused
    row-sum (ACT, fp16 out), reciprocal (DVE); normalization deferred
    to the output eviction.
  - out[d,s] = sum_e exp[e,d] vt[e,s]: 8 fp16 matmuls N=512, scaled by
    1/rowsum on the PSUM->SBUF eviction (DVE tensor_scalar, fp16 out),
    each 512-column block stored immediately (early drain).
"""

import math
import sys
from contextlib import ExitStack

for _p in ("/opt/trn_rl_repo", "/root/.axon_site/_ro/trn_rl_repo"):
    if _p not in sys.path:
        sys.path.insert(0, _p)

import numpy as np

import concourse.bacc as bacc
import concourse.bass as bass
import concourse.tile as tile
from concourse import mybir
from concourse.bass_utils import run_bass_kernel_spmd
from concourse.masks import make_identity

B, H, S, D = 4, 16, 4096, 128
NCORES = 8
PAIRS = (B * H) // NCORES  # 8 (b,h) pairs per core
NJ = S // 512              # 8 output column blocks of 512
SCALE = 1.0 / math.sqrt(H)
F32 = mybir.dt.float32
F16 = mybir.dt.float16


def _build():
    nc = bacc.Bacc(
        "TRN2",
        target_bir_lowering=False,
        debug=False,
        enable_asserts=False,
        num_devices=NCORES,
    )
    q = nc.dram_tensor("q", (PAIRS, S, D), F16, kind="ExternalInput").ap()
    k = nc.dram_tensor("k", (PAIRS, S, D), F16, kind="ExternalInput").ap()
    v = nc.dram_tensor("v", (PAIRS, S, D), F16, kind="ExternalInput").ap()
    out = nc.dram_tensor("out", (PAIRS, D, S), F16, kind="ExternalOutput").ap()

    with tile.TileContext(nc) as tc, ExitStack() as ctx:
        const = ctx.enter_context(tc.tile_pool(name="const", bufs=1))
        qkv = ctx.enter_context(tc.tile_pool(name="qkv", bufs=5))
        big = ctx.enter_context(tc.tile_pool(name="big", bufs=2))
        outp = ctx.enter_context(tc.tile_pool(name="outp", bufs=3))
        small = ctx.enter_context(tc.tile_pool(name="small", bufs=2))
        ps_score = ctx.enter_context(tc.tile_pool(name="ps_score", bufs=2, space="PSUM"))
        ps_vt = ctx.enter_context(tc.tile_pool(name="ps_vt", bufs=2, space="PSUM"))
        ps_pt = ctx.enter_context(tc.tile_pool(name="ps_pt", bufs=1, space="PSUM"))
        ps_out = ctx.enter_context(tc.tile_pool(name="ps_out", bufs=3, space="PSUM"))

        ident = const.tile([128, 128], F16)
        make_identity(nc, ident)

        for p in range(PAIRS):
            # ---- loads: (4096,128) f16 in halves; partition p_ holds rows
            # s = p_*32 + jj, jj = h*16 + j in [0,32).  The score contraction
            # is order-independent so any chunk order works as long as q and
            # k share the mapping; v's transpose un-permutes explicitly.
            last = p == PAIRS - 1
            q_sb = qkv.tile([128, 32, 128], F16, tag="q")
            k_sb = qkv.tile([128, 32, 128], F16, tag="k")
            v_sb = qkv.tile([128, 32, 128], F16, tag="v")
            if last:
                # the final pair's v goes first so its transposes clear the
                # PE pipe while q,k are still streaming in
                nc.sync.dma_start(
                    out=v_sb, in_=v[p].rearrange("(s j) d -> s j d", s=128)
                )
            nc.sync.dma_start(out=q_sb, in_=q[p].rearrange("(s j) d -> s j d", s=128))
            nc.sync.dma_start(out=k_sb, in_=k[p].rearrange("(s j) d -> s j d", s=128))
            if not last:
                nc.sync.dma_start(
                    out=v_sb, in_=v[p].rearrange("(s j) d -> s j d", s=128)
                )

            # ---- vT[e, s] via PE transpose (4 chunks per PSUM bank).
            # Transposing chunk jj yields [e, p_] columns for s = p_*32 + jj;
            # the copy-out un-permutes into true s-order with a stride-32
            # free-dim write: vt_sb layout is [e, p_, jj] so free pos = s.
            # ---- score[d,e] = sum_s q[s,d] k[s,e] ----
            # For the last pair the transposes are issued BEFORE score so the
            # post-load critical chain is just score -> softmax -> out.
            vt_sb = big.tile([128, 128, 32], F16, tag="vt")
            score_ps = ps_score.tile([128, 128], F32, tag="score")

            def vtrans_block():
                for g in range(8):
                    vt_ps = ps_vt.tile([128, 512], F16, tag="vt", name="vt_ps")
                    for i in range(4):
                        jj = 4 * g + i
                        nc.tensor.transpose(
                            vt_ps[:, 128 * i : 128 * (i + 1)], v_sb[:, jj, :], ident
                        )
                    nc.scalar.copy(
                        out=vt_sb[:, :, 4 * g : 4 * g + 4],
                        in_=vt_ps.rearrange("e (i s) -> e s i", i=4),
                    )

            def score_block():
                for jj in range(32):
                    nc.tensor.matmul(
                        score_ps,
                        q_sb[:, jj, :],
                        k_sb[:, jj, :],
                        start=(jj == 0),
                        stop=(jj == 31),
                    )

            if last:
                vtrans_block()
                score_block()
            else:
                score_block()
                vtrans_block()

            # ---- softmax over free axis e (normalization deferred) ----
            rowmax = small.tile([128, 1], F32, tag="rowmax")
            nc.vector.reduce_max(rowmax, score_ps, axis=mybir.AxisListType.X)
            negb = small.tile([128, 1], F32, tag="negb")
            nc.vector.tensor_scalar_mul(negb, rowmax, -SCALE)
            pexp = small.tile([128, 128], F16, tag="pexp")
            rowsum = small.tile([128, 1], F32, tag="rowsum")
            nc.scalar.activation(
                pexp,
                score_ps,
                mybir.ActivationFunctionType.Exp,
                bias=negb,
                scale=SCALE,
                accum_out=rowsum,
            )
            rinv = small.tile([128, 1], F32, tag="rinv")
            nc.vector.reciprocal(rinv, rowsum)

            # ---- pT[e,d] = exp(score)[d,e]^T ----
            pt_ps = ps_pt.tile([128, 128], F16, tag="pt")
            nc.tensor.transpose(pt_ps, pexp, ident)
            pt_sb = small.tile([128, 128], F16, tag="pt_sb")
            nc.vector.tensor_copy(out=pt_sb, in_=pt_ps)

            # ---- out[d,s] = (1/rowsum[d]) * sum_e pT[e,d] vt[e,s] ----
            # evictions scale by 1/rowsum; one 8 KiB/partition store on the
            # ACT engine's DGE ring so stores can't head-of-line-block loads.
            # The final pair stores in two halves so its store overlaps its
            # own tail evictions (everyone else keeps max-size descriptors).
            nhalf = 2 if p == PAIRS - 1 else 1
            orr = out[p].rearrange("d (u j s) -> u d j s", u=nhalf, j=NJ // nhalf)
            for u in range(nhalf):
                out_sb = outp.tile(
                    [128, NJ // nhalf, 512], F16, tag="out", name=f"out_sb{u}"
                )
                for jh in range(NJ // nhalf):
                    j = u * (NJ // nhalf) + jh
                    out_ps = ps_out.tile([128, 512], F32, tag="out")
                    nc.tensor.matmul(
                        out_ps,
                        pt_sb,
                        vt_sb[:, 16 * j : 16 * (j + 1), :],
                        start=True,
                        stop=True,
                    )
                    if last and j % 2 == 1:
                        # final pair: alternate evictions DVE/ACT to halve
                        # the trailing eviction chain (ACT is idle by then)
                        nc.scalar.activation(
                            out_sb[:, jh, :],
                            out_ps,
                            mybir.ActivationFunctionType.Copy,
                            scale=rinv,
                        )
                    else:
                        nc.vector.tensor_scalar_mul(out_sb[:, jh, :], out_ps, rinv)
                nc.scalar.dma_start(out=orr[u], in_=out_sb)

    nc.compile()
    return nc


_NC = None


def _get_nc():
    global _NC
    if _NC is None:
        _NC = _build()
    return _NC


def _in_maps(q, k, v):
    qf = np.ascontiguousarray(np.asarray(q).reshape(B * H, S, D).astype(np.float16))
    kf = np.ascontiguousarray(np.asarray(k).reshape(B * H, S, D).astype(np.float16))
    vf = np.ascontiguousarray(np.asarray(v).reshape(B * H, S, D).astype(np.float16))
    return [
        {
            "q": qf[i * PAIRS : (i + 1) * PAIRS],
            "k": kf[i * PAIRS : (i + 1) * PAIRS],
            "v": vf[i * PAIRS : (i + 1) * PAIRS],
        }
        for i in range(NCORES)
    ]


def _run(q, k, v, **kwargs):
    nc = _get_nc()
    res = run_bass_kernel_spmd(nc, _in_maps(q, k, v), core_ids=list(range(NCORES)), **kwargs)
    full = np.concatenate([res.results[i]["out"] for i in range(NCORES)], axis=0)
    return full.astype(np.float32).reshape(B, H, D, S), res


def kernel(q, k, v):
    out, _ = _run(q, k, v)
    return out


# revision 30
# speedup vs baseline: 1.1204x; 1.0805x over previous
"""Trainium2 Bass kernel: feature-attention (dense_transformer).

    score = softmax((q^T @ k) / sqrt(H), axis=-1)   # (B,H,D,D), contraction over S
    out   = score @ v^T                              # (B,H,D,S)

q,k,v: (4,16,4096,128) f32.  B*H = 64 head-pairs sharded 8-per-core across
8 NeuronCores (pure data/head parallelism, no collectives).

The kernel is DMA-bound (fp32 would be 64 MiB/core @ 360 GB/s = 187 us
floor), so all device I/O is fp16: inputs are downcast on the host
(logit error ~ sqrt(S)*2^-11*scale stays ~1e-3, final rel err ~6e-3 vs
the 2e-2 gate), output is written fp16 and upcast on the host.  Traffic
is 32 MiB/core -> ~93 us DMA floor.  fp16 matmuls run at 1 cyc/row.

Per (b,h) pair on-core:
  - q,k,v loaded as [128, 16, 128] halves (seq chunked onto partitions,
    4 KiB/partition contiguous) so score matmuls start after the first
    0.5 MiB lands and DMA dependencies stay fine-grained.
  - score[d,e] = sum_s q[s,d] k[s,e]: 32 accumulating fp16 PE matmuls
    (fp32 PSUM) into one PSUM tile.
  - v^T via PE transpose-mode matmuls (fp16, 1 cyc/row), 4 chunks per
    PSUM bank; PSUM->SBUF copies alternate between ACT and DVE to
    balance engine load.
  - softmax along free axis in fp32: reduce_max (DVE), exp with fused
    row-sum (ACT, fp16 out), reciprocal (DVE); normalization deferred
    to the output eviction.
  - out[d,s] = sum_e exp[e,d] vt[e,s]: 8 fp16 matmuls N=512, scaled by
    1/rowsum on the PSUM->SBUF eviction (DVE tensor_scalar, fp16 out),
    each 512-column block stored immediately (early drain).
"""

import math
import sys
from contextlib import ExitStack

for _p in ("/opt/trn_rl_repo", "/root/.axon_site/_ro/trn_rl_repo"):
    if _p not in sys.path:
        sys.path.insert(0, _p)

import numpy as np

import concourse.bacc as bacc
import concourse.bass as bass
import concourse.tile as tile
from concourse import mybir
from concourse.bass_utils import run_bass_kernel_spmd
from concourse.masks import make_identity

B, H, S, D = 4, 16, 4096, 128
NCORES = 8
PAIRS = (B * H) // NCORES  # 8 (b,h) pairs per core
NJ = S // 512              # 8 output column blocks of 512
SCALE = 1.0 / math.sqrt(H)
F32 = mybir.dt.float32
F16 = mybir.dt.float16


def _build():
    nc = bacc.Bacc(
        "TRN2",
        target_bir_lowering=False,
        debug=False,
        enable_asserts=False,
        num_devices=NCORES,
    )
    q = nc.dram_tensor("q", (PAIRS, S, D), F16, kind="ExternalInput").ap()
    k = nc.dram_tensor("k", (PAIRS, S, D), F16, kind="ExternalInput").ap()
    v = nc.dram_tensor("v", (PAIRS, S, D), F16, kind="ExternalInput").ap()
    out = nc.dram_tensor("out", (PAIRS, D, S), F16, kind="ExternalOutput").ap()

    with tile.TileContext(nc) as tc, ExitStack() as ctx:
        const = ctx.enter_context(tc.tile_pool(name="const", bufs=1))
        qkv = ctx.enter_context(tc.tile_pool(name="qkv", bufs=5))
        big = ctx.enter_context(tc.tile_pool(name="big", bufs=2))
        outp = ctx.enter_context(tc.tile_pool(name="outp", bufs=3))
        small = ctx.enter_context(tc.tile_pool(name="small", bufs=2))
        ps_score = ctx.enter_context(tc.tile_pool(name="ps_score", bufs=2, space="PSUM"))
        ps_vt = ctx.enter_context(tc.tile_pool(name="ps_vt", bufs=2, space="PSUM"))
        ps_pt = ctx.enter_context(tc.tile_pool(name="ps_pt", bufs=1, space="PSUM"))
        ps_out = ctx.enter_context(tc.tile_pool(name="ps_out", bufs=3, space="PSUM"))

        ident = const.tile([128, 128], F16)
        make_identity(nc, ident)

        for p in range(PAIRS):
            # ---- loads: (4096,128) f16 in halves; partition p_ holds rows
            # s = p_*32 + jj, jj = h*16 + j in [0,32).  The score contraction
            # is order-independent so any chunk order works as long as q and
            # k share the mapping; v's transpose un-permutes explicitly.
            last = p == PAIRS - 1
            q_sb = qkv.tile([128, 32, 128], F16, tag="q")
            k_sb = qkv.tile([128, 32, 128], F16, tag="k")
            v_sb = qkv.tile([128, 32, 128], F16, tag="v")
            if last:
                # the final pair's v goes first so its transposes clear the
                # PE pipe while q,k are still streaming in
                nc.sync.dma_start(
                    out=v_sb, in_=v[p].rearrange("(s j) d -> s j d", s=128)
                )
            nc.sync.dma_start(out=q_sb, in_=q[p].rearrange("(s j) d -> s j d", s=128))
            nc.sync.dma_start(out=k_sb, in_=k[p].rearrange("(s j) d -> s j d", s=128))
            if not last:
                nc.sync.dma_start(
                    out=v_sb, in_=v[p].rearrange("(s j) d -> s j d", s=128)
                )

            # ---- vT[e, s] via PE transpose (4 chunks per PSUM bank).
            # Transposing chunk jj yields [e, p_] columns for s = p_*32 + jj;
            # the copy-out un-permutes into true s-order with a stride-32
            # free-dim write: vt_sb layout is [e, p_, jj] so free pos = s.
            # ---- score[d,e] = sum_s q[s,d] k[s,e] ----
            # For the last pair the transposes are issued BEFORE score so the
            # post-load critical chain is just score -> softmax -> out.
            vt_sb = big.tile([128, 128, 32], F16, tag="vt")
            score_ps = ps_score.tile([128, 128], F32, tag="score")

            def vtrans_block():
                for g in range(8):
                    vt_ps = ps_vt.tile([128, 512], F16, tag="vt", name="vt_ps")
                    for i in range(4):
                        jj = 4 * g + i
                        nc.tensor.transpose(
                            vt_ps[:, 128 * i : 128 * (i + 1)], v_sb[:, jj, :], ident
                        )
                    nc.scalar.copy(
                        out=vt_sb[:, :, 4 * g : 4 * g + 4],
                        in_=vt_ps.rearrange("e (i s) -> e s i", i=4),
                    )

            def score_block():
                for jj in range(32):
                    nc.tensor.matmul(
                        score_ps,
                        q_sb[:, jj, :],
                        k_sb[:, jj, :],
                        start=(jj == 0),
                        stop=(jj == 31),
                    )

            if last:
                vtrans_block()
                score_block()
            else:
                score_block()
                vtrans_block()

            # ---- softmax over free axis e (normalization deferred) ----
            rowmax = small.tile([128, 1], F32, tag="rowmax")
            nc.vector.reduce_max(rowmax, score_ps, axis=mybir.AxisListType.X)
            negb = small.tile([128, 1], F32, tag="negb")
            nc.vector.tensor_scalar_mul(negb, rowmax, -SCALE)
            pexp = small.tile([128, 128], F16, tag="pexp")
            rowsum = small.tile([128, 1], F32, tag="rowsum")
            nc.scalar.activation(
                pexp,
                score_ps,
                mybir.ActivationFunctionType.Exp,
                bias=negb,
                scale=SCALE,
                accum_out=rowsum,
            )
            rinv = small.tile([128, 1], F32, tag="rinv")
            nc.vector.reciprocal(rinv, rowsum)

            # ---- pT[e,d] = exp(score)[d,e]^T ----
            pt_ps = ps_pt.tile([128, 128], F16, tag="pt")
            nc.tensor.transpose(pt_ps, pexp, ident)
            pt_sb = small.tile([128, 128], F16, tag="pt_sb")
            nc.scalar.copy(out=pt_sb, in_=pt_ps)

            # ---- out[d,s] = (1/rowsum[d]) * sum_e pT[e,d] vt[e,s] ----
            # evictions scale by 1/rowsum; one 8 KiB/partition store on the
            # ACT engine's DGE ring so stores can't head-of-line-block loads.
            # The final pair stores in two halves so its store overlaps its
            # own tail evictions (everyone else keeps max-size descriptors).
            nhalf = 2 if p == PAIRS - 1 else 1
            orr = out[p].rearrange("d (u j s) -> u d j s", u=nhalf, j=NJ // nhalf)
            for u in range(nhalf):
                out_sb = outp.tile(
                    [128, NJ // nhalf, 512], F16, tag="out", name=f"out_sb{u}"
                )
                for jh in range(NJ // nhalf):
                    j = u * (NJ // nhalf) + jh
                    out_ps = ps_out.tile([128, 512], F32, tag="out")
                    nc.tensor.matmul(
                        out_ps,
                        pt_sb,
                        vt_sb[:, 16 * j : 16 * (j + 1), :],
                        start=True,
                        stop=True,
                    )
                    if last and j % 2 == 1:
                        # final pair: alternate evictions DVE/ACT to halve
                        # the trailing eviction chain (ACT is idle by then)
                        nc.scalar.activation(
                            out_sb[:, jh, :],
                            out_ps,
                            mybir.ActivationFunctionType.Copy,
                            scale=rinv,
                        )
                    else:
                        nc.vector.tensor_scalar_mul(out_sb[:, jh, :], out_ps, rinv)
                nc.scalar.dma_start(out=orr[u], in_=out_sb)

    nc.compile()
    return nc


_NC = None


def _get_nc():
    global _NC
    if _NC is None:
        _NC = _build()
    return _NC


def _in_maps(q, k, v):
    qf = np.ascontiguousarray(np.asarray(q).reshape(B * H, S, D).astype(np.float16))
    kf = np.ascontiguousarray(np.asarray(k).reshape(B * H, S, D).astype(np.float16))
    vf = np.ascontiguousarray(np.asarray(v).reshape(B * H, S, D).astype(np.float16))
    return [
        {
            "q": qf[i * PAIRS : (i + 1) * PAIRS],
            "k": kf[i * PAIRS : (i + 1) * PAIRS],
            "v": vf[i * PAIRS : (i + 1) * PAIRS],
        }
        for i in range(NCORES)
    ]


def _run(q, k, v, **kwargs):
    nc = _get_nc()
    res = run_bass_kernel_spmd(nc, _in_maps(q, k, v), core_ids=list(range(NCORES)), **kwargs)
    full = np.concatenate([res.results[i]["out"] for i in range(NCORES)], axis=0)
    return full.astype(np.float32).reshape(B, H, D, S), res


def kernel(q, k, v):
    out, _ = _run(q, k, v)
    return out
